# revision 12
# baseline (speedup 1.0000x reference)
"""Trainium2 Bass kernel for nn_CrowdsClassificationSModel.

Reference computation:
    W = softmax(kernel, axis=1)            # (8, 8, 59)
    out = einsum('bc,cdr->bdr', x, W)      # (131072, 8, 59)
    out = where(drop_mask, out / 0.6, 0)

Data-parallel over 8 NeuronCores (batch-sharded, bc=16384 per core).
Per core, batch b = p*128 + n for SBUF/PSUM partition p, n in [0,128);
PSUM tile t covers n = 4t+q, q in [0,4).

Device-side design (layout-heavy work precomputed on host):

  - lhsT is built on HOST: x is bf16-split (xh + xl ~= x) and laid out
    as matmul stationary blocks [K=TERMS*8, 128] per batch-group n
    (contraction over c only — no PE transposes, no zero-padding).
    The 128 blocks are stacked 4-deep across SBUF partitions (block
    for group g lives at partitions 32*(g%4)..) so the single load
    engages all 16 DMA engines; W is replicated at the 4 quadrant
    bases to match (PE tile_position rows 0/32/64/96).
  - rhs is the shared W block [K, 472]: rows [Wh; Wh; Wl] so
    out = xh@Wh + xl@Wh + xh@Wl (error ~2^-18), /0.6 folded in.
  - Per PSUM tile t: 4 matmuls (q=0..3) write [128, 472] each into one
    4-bank [128, 2048] PSUM tile; ONE fused DVE op applies the
    dropout mask across all 1888 elements via strided views.
  - KMASK=u8: mask bytes 0/1, plain tensor_mul.
    KMASK=2bit: 2 mask bits per byte (tile pair 2T,2T+1 -> v=m0+2*m1),
    extracted arithmetically in the fused scalar_tensor_tensor:
    tile 2T uses (v mod 2) * pm, tile 2T+1 uses (v >= 2) * pm.
  - KOUT=bf16 writes bf16 to DRAM and the host widens to f32 (adds
    ~2e-3 max rel err vs the 2e-2 gate); KOUT=f32 keeps full fidelity.
  - KGP>0 offloads tiles t%KGP==1 to gpsimd (ScalarE copies PSUM->SBUF
    first, since gpsimd has no PSUM port) to relieve the DVE.
"""

import os as _os

import numpy as np

import concourse.bacc as bacc
import concourse.bass as bass
import concourse.tile as tile
from concourse import mybir
from concourse.bass_utils import run_bass_kernel_spmd

N_CORES = 8
B_FULL = 131072
C = 8
R = 59
F = C * R  # 472
DROP_RATE = 0.4
KEEP = np.float32(1.0 - DROP_RATE)

TERMS = int(_os.environ.get("KTERMS", "3"))  # bf16 product terms (1..3)
KOUT = _os.environ.get("KOUT", "bf16")  # "f32" | "bf16"
KMASK = _os.environ.get("KMASK", "u8")  # "u8" | "2bit"
GP_EVERY = int(_os.environ.get("KGP", "3"))  # gpsimd handles t%GP_EVERY==1

NQ = 4  # batch groups per PSUM tile
NT = 128 // NQ  # 32 psum tiles per core
K_DIM = TERMS * C
FS = NQ * F  # 1888


def softmax_np(k: np.ndarray, axis: int) -> np.ndarray:
    k = k.astype(np.float64)
    m = k.max(axis=axis, keepdims=True)
    e = np.exp(k - m)
    return (e / e.sum(axis=axis, keepdims=True)).astype(np.float32)


def build_w(kernel: np.ndarray) -> np.ndarray:
    """(8,8,59) raw -> [K_DIM, 472] bf16 rhs rows [Wh; Wh; Wl]."""
    import ml_dtypes

    w = softmax_np(kernel, axis=1).reshape(C, F) / KEEP  # (8, 472) f32
    wh = w.astype(ml_dtypes.bfloat16)
    wl = (w - wh.astype(np.float32)).astype(ml_dtypes.bfloat16)
    return np.concatenate(((wh, wh, wl)[:TERMS]), axis=0)  # (K_DIM, 472)


def build_lhst(x_sh: np.ndarray) -> np.ndarray:
    """(16384, 8) f32 -> [K_DIM, 128*128] bf16 stationary blocks.

    lhsT[(t, c), n*128 + m] = split_t(x[m*128 + n, c])
    """
    import ml_dtypes

    xs = x_sh.reshape(128, 128, C)  # [m, n, c]
    xh = xs.astype(ml_dtypes.bfloat16)
    xl = (xs - xh.astype(np.float32)).astype(ml_dtypes.bfloat16)
    arr = np.stack((xh, xl, xh)[:TERMS], axis=0)  # [t, m, n, c]
    return np.ascontiguousarray(
        arr.transpose(0, 3, 2, 1).reshape(K_DIM, 128 * 128)
    )


def pack_mask(mask_sh: np.ndarray) -> np.ndarray:
    """(16384, 472) u8 -> per KMASK layout."""
    if KMASK == "u8":
        return np.ascontiguousarray(mask_sh)
    # 2bit: byte[p, T, q, f] = m[p, 8T+q, f] + 2*m[p, 8T+4+q, f]
    mk = mask_sh.reshape(128, NT // 2, 2, NQ, F)  # [p, T, h, q, f]
    v = mk[:, :, 0] + 2 * mk[:, :, 1]  # [p, T, q, f]
    return np.ascontiguousarray(v.reshape(128, (NT // 2) * FS))


def build_module(bc: int) -> bass.Bass:
    assert bc == 128 * 128
    nc = bacc.Bacc("TRN2", target_bir_lowering=False, debug=False)
    f32 = mybir.dt.float32
    bf16 = mybir.dt.bfloat16
    u8 = mybir.dt.uint8
    odt = f32 if KOUT == "f32" else bf16

    xt_d = nc.dram_tensor(
        "xt_blk", (K_DIM, 128 * 128), bf16, kind="ExternalInput"
    )
    w_d = nc.dram_tensor("w_blk", (K_DIM, F), bf16, kind="ExternalInput")
    if KMASK == "u8":
        m_d = nc.dram_tensor("mask_sh", (bc, F), u8, kind="ExternalInput")
        m_view = m_d[:].rearrange("(p t q) f -> t p (q f)", p=128, t=NT, q=NQ)
    else:
        m_d = nc.dram_tensor(
            "mask_sh", (128, (NT // 2) * FS), u8, kind="ExternalInput"
        )
        m_view = m_d[:].rearrange("p (T z) -> T p z", T=NT // 2)
    o_d = nc.dram_tensor("out_sh", (bc, F), odt, kind="ExternalOutput")
    o_view = o_d[:].rearrange("(p t q) f -> t p (q f)", p=128, t=NT, q=NQ)

    with tile.TileContext(nc) as tc:
        with (
            tc.tile_pool(name="const", bufs=1) as constp,
            tc.tile_pool(name="mask", bufs=6) as maskp,
            tc.tile_pool(name="stage", bufs=6) as stagep,
            tc.tile_pool(name="pmm", bufs=2, space="PSUM") as pmmp,
            tc.tile_pool(name="sb", bufs=2) as sbp,
        ):
            # lhsT is only 24 partitions wide -> a single load would run on
            # 3 of 16 DMA engines (~10 us). Chunk it so chunk 0 lands fast
            # and the rest stream behind compute.
            N_CHUNK = 8
            CW = (128 * 128) // N_CHUNK
            xt_chunks = []
            for ch in range(N_CHUNK):
                xc = constp.tile([K_DIM, CW], bf16)
                nc.scalar.dma_start(xc[:], xt_d[:, ch * CW : (ch + 1) * CW])
                xt_chunks.append(xc)
            w_t = constp.tile([K_DIM, F], bf16)
            nc.scalar.dma_start(w_t[:], w_d[:])
            tiles_per_chunk = NT // N_CHUNK

            mt = None
            for t in range(NT):
                if KMASK == "u8":
                    mt = maskp.tile([128, FS], u8)
                    nc.scalar.dma_start(mt[:], m_view[t])
                elif t % 2 == 0:
                    mt = maskp.tile([128, FS], u8)
                    nc.scalar.dma_start(mt[:], m_view[t // 2])

                pm = pmmp.tile([128, NQ * 512], f32)
                xc = xt_chunks[t // tiles_per_chunk]
                tl = t % tiles_per_chunk
                for q in range(NQ):
                    g = NQ * tl + q
                    nc.tensor.matmul(
                        pm[:, q * 512 : q * 512 + F],
                        xc[:, g * 128 : (g + 1) * 128],
                        w_t[:],
                        start=True,
                        stop=True,
                    )
                pmv = pm[:].rearrange("p (q z) -> p q z", q=NQ)[:, :, 0:F]
                mkv = mt[:].rearrange("p (q z) -> p q z", q=NQ)
                st = stagep.tile([128, FS], odt)
                stv = st[:].rearrange("p (q z) -> p q z", q=NQ)
                use_gp = GP_EVERY and (t % GP_EVERY == 1)
                eng = nc.gpsimd if use_gp else nc.vector
                if use_gp:
                    sb = sbp.tile([128, FS], f32)
                    sbv = sb[:].rearrange("p (q z) -> p q z", q=NQ)
                    nc.scalar.copy(sbv, pmv)
                    src = sbv
                else:
                    src = pmv
                if KMASK == "u8":
                    eng.tensor_mul(stv, src, mkv)
                else:
                    op0 = (
                        mybir.AluOpType.mod
                        if t % 2 == 0
                        else mybir.AluOpType.is_ge
                    )
                    eng.scalar_tensor_tensor(
                        stv, mkv, 2, src, op0=op0, op1=mybir.AluOpType.mult
                    )
                nc.sync.dma_start(o_view[t], st[:])

    nc.compile()
    return nc


_CACHE: dict = {}


def _get_module(bc: int):
    key = (bc, TERMS, KOUT, KMASK, GP_EVERY)
    if key not in _CACHE:
        _CACHE[key] = build_module(bc)
    return _CACHE[key]


def _prep_inputs(x, kernel, drop_mask, bc):
    w_blk = build_w(np.asarray(kernel))
    x = np.ascontiguousarray(np.asarray(x, dtype=np.float32))
    mask = np.asarray(drop_mask)
    if mask.dtype != np.uint8:
        mask = mask.astype(np.uint8)
    mask = np.ascontiguousarray(mask.reshape(mask.shape[0], -1))
    n_shards = x.shape[0] // bc
    in_maps = []
    for i in range(n_shards):
        in_maps.append(
            {
                "xt_blk": build_lhst(x[i * bc : (i + 1) * bc]),
                "w_blk": w_blk,
                "mask_sh": pack_mask(mask[i * bc : (i + 1) * bc]),
            }
        )
    return in_maps


def run(x, kernel, drop_mask, trace: bool = False):
    bc = x.shape[0] // N_CORES
    nc = _get_module(bc)
    in_maps = _prep_inputs(x, kernel, drop_mask, bc)
    res = run_bass_kernel_spmd(
        nc, in_maps, core_ids=list(range(N_CORES)), trace=trace
    )
    out = np.concatenate(
        [np.asarray(r["out_sh"], dtype=np.float32) for r in res.results], axis=0
    )
    return out.reshape(B_FULL, C, R), res


def kernel(x, kernel, drop_mask) -> np.ndarray:
    out, _ = run(x, kernel, drop_mask, trace=False)
    return out
